# revision 4
# baseline (speedup 1.0000x reference)
"""CFConv fused GNN message-passing kernel for 8 Trainium2 NeuronCores.

Strategy (edge-parallel, dst-sharded):
- Host sorts edges by dst and buckets them to 8 cores by dst range (12500
  nodes/core). Within a core, edges are grouped by 128-node dst-groups and
  padded to 128-edge chunks. As part of sharding, the host gathers x[src]
  into the per-edge layout (transposed, bias-row-augmented).
- Each core, per chunk of 128 edges: two matmuls compute the pre-linear
  (hv) and radial filter (filt) into PSUM; DVE multiplies them into msg;
  DVE builds a one-hot(dst) matrix; a third matmul accumulates
  onehot.T @ msg into the group's PSUM h (segment sum). Per group, h is
  transposed on PE, augmented with a ones row, multiplied by W_post
  (bias folded), passed through SiLU on the scalar engine, and written out
  transposed. Host concatenates and transposes the output.
"""
import sys
sys.path.insert(0, "/opt/trn_rl_repo")
import os
import numpy as np

N_NODES = 100000
N_EDGES = 1600000
D_IN = 64
D_RAD = 50
D_H = 64
D_OUT = 64
N_CORES = 8
NPC = N_NODES // N_CORES          # nodes per core
P = 128
NG = (NPC + P - 1) // P           # dst groups per core (98)

_EXEC = {}


def _prep(x, edge_basis, src, dst, dt_np):
    """Host-side sharding: returns per-core input dicts + program shape info."""
    order = np.argsort(dst, kind="stable")
    dst_s = dst[order].astype(np.int64)
    src_s = src[order].astype(np.int64)

    core_bounds = np.searchsorted(dst_s, np.arange(N_CORES + 1) * NPC)
    # per-(core, group) counts
    counts = np.zeros((N_CORES, NG), dtype=np.int64)
    core_edges = []
    for c in range(N_CORES):
        lo, hi = core_bounds[c], core_bounds[c + 1]
        e = order[lo:hi]
        d_rel = dst_s[lo:hi] - c * NPC
        g = d_rel // P
        counts[c] = np.bincount(g, minlength=NG)
        core_edges.append((e, d_rel, src_s[lo:hi], g))

    chunks_g = np.maximum(1, (counts.max(axis=0) + P - 1) // P)   # per group
    offs = np.concatenate([[0], np.cumsum(chunks_g)])             # chunk offsets
    SC = int(offs[-1])                                            # total chunks
    S = SC * P                                                    # total edge slots

    in_maps = []
    for c in range(N_CORES):
        e, d_rel, s_ids, g = core_edges[c]
        gstart = np.concatenate([[0], np.cumsum(counts[c])])
        rank = np.arange(len(e)) - gstart[g]                      # rank within group
        slot = (offs[g] * P + rank).astype(np.int64)

        xg = np.zeros((S, D_IN + 1), dtype=np.float32)
        xg[slot, :D_IN] = x[s_ids]
        xg[:, D_IN] = 1.0
        xgT = np.ascontiguousarray(xg.T).astype(dt_np)

        bs = np.zeros((S, D_RAD + 1), dtype=np.float32)
        bs[slot, :D_RAD] = edge_basis[e]
        bs[:, D_RAD] = 1.0
        bsT = np.ascontiguousarray(bs.T).astype(dt_np)

        dstrel = np.full((P, SC), -1.0, dtype=np.float32)
        dstrel[slot % P, slot // P] = (d_rel - g * P).astype(np.float32)
        in_maps.append({"xgT": xgT, "bsT": bsT, "dstrel": dstrel.astype(dt_np)})
    return in_maps, chunks_g, offs, SC, S


def _build(chunks_g, offs, SC, S, dt, ng_limit=None):
    from concourse import bass, bacc, mybir, tile
    from concourse.masks import make_identity
    f32 = mybir.dt.float32
    ng = NG if ng_limit is None else ng_limit

    nc = bacc.Bacc(None, target_bir_lowering=False)
    xgT = nc.dram_tensor("xgT", [D_IN + 1, S], dt, kind="ExternalInput")
    bsT = nc.dram_tensor("bsT", [D_RAD + 1, S], dt, kind="ExternalInput")
    dstrel = nc.dram_tensor("dstrel", [P, SC], dt, kind="ExternalInput")
    iota_in = nc.dram_tensor("iota_in", [P, P], dt, kind="ExternalInput")
    wpre = nc.dram_tensor("wpre", [D_IN + 1, D_H], dt, kind="ExternalInput")
    wrad = nc.dram_tensor("wrad", [D_RAD + 1, D_H], dt, kind="ExternalInput")
    wpost = nc.dram_tensor("wpost", [D_H + 1, D_OUT], dt, kind="ExternalInput")
    outT = nc.dram_tensor("outT", [D_OUT, NPC], f32, kind="ExternalOutput")

    BATCH = 4
    with tile.TileContext(nc) as tc:
        with (
            tc.tile_pool(name="const", bufs=1) as const,
            tc.tile_pool(name="io", bufs=4) as io,
            tc.tile_pool(name="work", bufs=3) as work,
            tc.tile_pool(name="ep", bufs=2) as ep,
            tc.tile_pool(name="ps_pair", bufs=2, space="PSUM") as ps_pair,
            tc.tile_pool(name="ps_h", bufs=2, space="PSUM") as ps_h,
            tc.tile_pool(name="ps_ep", bufs=2, space="PSUM") as ps_ep,
        ):
            iota_t = const.tile([P, P], dt, name="iota_t")
            nc.sync.dma_start(iota_t[:], iota_in[:])
            ident = const.tile([P, P], dt, name="ident")
            make_identity(nc, ident[:])
            wpre_t = const.tile([D_IN + 1, D_H], dt, name="wpre_t")
            nc.sync.dma_start(wpre_t[:], wpre[:])
            wrad_t = const.tile([D_RAD + 1, D_H], dt, name="wrad_t")
            nc.sync.dma_start(wrad_t[:], wrad[:])
            wpost_t = const.tile([D_H + 1, D_OUT], dt, name="wpost_t")
            nc.sync.dma_start(wpost_t[:], wpost[:])
            dst_res = const.tile([P, SC], dt, name="dst_res")
            nc.sync.dma_start(dst_res[:], dstrel[:])

            for g in range(ng):
                n_in_g = min(P, NPC - g * P)
                nch = int(chunks_g[g])
                j0 = int(offs[g])
                h_ps = ps_h.tile([P, D_H], f32, tag="h_ps", name="h_ps")
                kglob = 0
                for b0 in range(0, nch, BATCH):
                    B = min(BATCH, nch - b0)
                    cs = (j0 + b0) * P
                    xg_t = io.tile([D_IN + 1, BATCH * P], dt, tag="xg", name="xg_t")
                    nc.sync.dma_start(xg_t[:, : B * P], xgT[:, cs : cs + B * P])
                    bs_t = io.tile([D_RAD + 1, BATCH * P], dt, tag="bs", name="bs_t")
                    nc.sync.dma_start(bs_t[:, : B * P], bsT[:, cs : cs + B * P])

                    pp = ps_pair.tile([P, BATCH, P], f32, tag="pp", name="pp")
                    for k in range(B):
                        nc.tensor.matmul(
                            pp[:, k, 0:D_H],
                            lhsT=xg_t[:, k * P : (k + 1) * P],
                            rhs=wpre_t[:],
                            start=(k == 0), stop=False,
                        )
                        nc.tensor.matmul(
                            pp[:, k, D_H : D_H + D_H],
                            lhsT=bs_t[:, k * P : (k + 1) * P],
                            rhs=wrad_t[:],
                            start=False, stop=(k == B - 1),
                        )
                    hv_sb = work.tile([P, BATCH, D_H], dt, tag="hv_sb", name="hv_sb")
                    nc.scalar.copy(hv_sb[:, :B, :], pp[:, :B, 0:D_H])
                    msg = work.tile([P, BATCH, D_H], dt, tag="msg", name="msg")
                    nc.vector.tensor_tensor(
                        out=msg[:, :B, :],
                        in0=hv_sb[:, :B, :],
                        in1=pp[:, :B, D_H : 2 * D_H],
                        op=mybir.AluOpType.mult,
                    )
                    oh = work.tile([P, BATCH * P], dt, tag="oh", name="oh")
                    for k in range(B):
                        nc.vector.tensor_tensor(
                            out=oh[:, k * P : (k + 1) * P],
                            in0=dst_res[:, j0 + b0 + k : j0 + b0 + k + 1].to_broadcast([P, P]),
                            in1=iota_t[:],
                            op=mybir.AluOpType.is_equal,
                        )
                    for k in range(B):
                        nc.tensor.matmul(
                            h_ps[:],
                            lhsT=oh[:, k * P : (k + 1) * P],
                            rhs=msg[:, k, :],
                            start=(kglob == 0), stop=(kglob == nch - 1),
                        )
                        kglob += 1

                # epilogue: out[:, g*P : g*P+n] = silu(Wpost_aug.T @ [h.T; 1])
                h_sb = ep.tile([P, D_H], dt, tag="h_sb", name="h_sb")
                nc.vector.tensor_copy(h_sb[:], h_ps[:])
                ht_ps = ps_ep.tile([D_H, P], dt, tag="ht_ps", name="ht_ps")
                nc.tensor.transpose(ht_ps[:], h_sb[:], ident[:])
                haug = ep.tile([D_H + 1, P], dt, tag="haug", name="haug")
                nc.vector.tensor_copy(haug[:D_H, :], ht_ps[:])
                nc.vector.memset(haug[D_H : D_H + 1, :], 1.0)
                o_ps = ps_ep.tile([D_OUT, P], f32, tag="o_ps", name="o_ps")
                nc.tensor.matmul(o_ps[:], lhsT=wpost_t[:], rhs=haug[:], start=True, stop=True)
                o_sb = ep.tile([D_OUT, P], f32, tag="o_sb", name="o_sb")
                nc.scalar.activation(o_sb[:], o_ps[:], mybir.ActivationFunctionType.Silu)
                nc.sync.dma_start(outT[:, g * P : g * P + n_in_g], o_sb[:, :n_in_g])
    nc.compile()
    return nc


class _Exec:
    """Build-once PJRT executor (shard_map over 8 cores)."""

    def __init__(self, nc, n_cores):
        import jax
        from jax.sharding import Mesh, PartitionSpec, NamedSharding
        from jax.experimental.shard_map import shard_map
        from concourse import mybir, bass2jax
        from concourse.bass2jax import _bass_exec_p, install_neuronx_cc_hook

        install_neuronx_cc_hook()
        self.jax = jax
        self.n_cores = n_cores
        partition_name = nc.partition_id_tensor.name if nc.partition_id_tensor else None
        in_names, out_names, out_avals, self.zero_shapes = [], [], [], []
        for alloc in nc.m.functions[0].allocations:
            if not isinstance(alloc, mybir.MemoryLocationSet):
                continue
            name = alloc.memorylocations[0].name
            if alloc.kind == "ExternalInput":
                if name != partition_name:
                    in_names.append(name)
            elif alloc.kind == "ExternalOutput":
                shape = tuple(alloc.tensor_shape)
                dtype = mybir.dt.np(alloc.dtype)
                out_names.append(name)
                out_avals.append(jax.core.ShapedArray(shape, dtype))
                self.zero_shapes.append((shape, dtype))
        self.in_names, self.out_names, self.out_avals = in_names, out_names, out_avals
        n_params, n_outs = len(in_names), len(out_avals)
        all_in = in_names + out_names + ([partition_name] if partition_name else [])

        def _body(*args):
            operands = list(args)
            if partition_name is not None:
                operands.append(bass2jax.partition_id_tensor())
            return tuple(_bass_exec_p.bind(
                *operands,
                out_avals=tuple(out_avals),
                in_names=tuple(all_in),
                out_names=tuple(out_names),
                lowering_input_output_aliases=(),
                sim_require_finite=True,
                sim_require_nnan=True,
                nc=nc,
            ))

        devices = jax.devices()[:n_cores]
        self.mesh = Mesh(np.asarray(devices), ("core",))
        self.fn = jax.jit(
            shard_map(_body, mesh=self.mesh,
                      in_specs=(PartitionSpec("core"),) * (n_params + n_outs),
                      out_specs=(PartitionSpec("core"),) * n_outs,
                      check_rep=False),
            donate_argnums=tuple(range(n_params, n_params + n_outs)),
            keep_unused=True,
        )
        self.sharding = NamedSharding(self.mesh, PartitionSpec("core"))

    def put_inputs(self, in_maps):
        return [self.jax.device_put(
                    np.concatenate([np.asarray(m[n]) for m in in_maps], axis=0),
                    self.sharding)
                for n in self.in_names]

    def zeros(self):
        return [self.jax.device_put(
                    np.zeros((self.n_cores * s[0], *s[1:]), d), self.sharding)
                for s, d in self.zero_shapes]

    def run(self, dev_inputs):
        outs = self.fn(*dev_inputs, *self.zeros())
        self.jax.block_until_ready(outs)
        mats = [np.asarray(o) for o in outs]
        return [
            {n: mats[i].reshape(self.n_cores, *self.out_avals[i].shape)[c]
             for i, n in enumerate(self.out_names)}
            for c in range(self.n_cores)
        ]


def _get_exec(x, edge_basis, src, dst, W_pre, b_pre, W_rad, b_rad, W_post, b_post,
              dt_name=None, ng_limit=None):
    from concourse import mybir
    dt_name = dt_name or os.environ.get("CFCONV_DT", "float16")
    dt = {"float32": mybir.dt.float32, "float16": mybir.dt.float16,
          "bfloat16": mybir.dt.bfloat16}[dt_name]
    dt_np = {"float32": np.float32, "float16": np.float16,
             "bfloat16": np.float32}[dt_name]  # bf16 arrays passed as f32? avoid bf16

    in_maps, chunks_g, offs, SC, S = _prep(x, edge_basis, src, dst, dt_np)
    key = (dt_name, SC, S, tuple(chunks_g), ng_limit)
    if key not in _EXEC:
        nc = _build(chunks_g, offs, SC, S, dt, ng_limit)
        _EXEC[key] = _Exec(nc, N_CORES)
    ex = _EXEC[key]

    iota = np.tile(np.arange(P, dtype=np.float32), (P, 1)).astype(dt_np)
    wpre_np = np.vstack([W_pre.T, b_pre[None, :]]).astype(dt_np)
    wrad_np = np.vstack([W_rad.T, b_rad[None, :]]).astype(dt_np)
    wpost_np = np.vstack([W_post.T, b_post[None, :]]).astype(dt_np)
    for m in in_maps:
        m["iota_in"] = iota
        m["wpre"] = wpre_np
        m["wrad"] = wrad_np
        m["wpost"] = wpost_np
    return ex, in_maps


def kernel(x, edge_basis, src, dst, W_pre, b_pre, W_rad, b_rad, W_post, b_post):
    x = np.asarray(x, dtype=np.float32)
    edge_basis = np.asarray(edge_basis, dtype=np.float32)
    src = np.asarray(src)
    dst = np.asarray(dst)
    ex, in_maps = _get_exec(x, edge_basis, np.asarray(src, np.int32),
                            np.asarray(dst, np.int32),
                            np.asarray(W_pre, np.float32), np.asarray(b_pre, np.float32),
                            np.asarray(W_rad, np.float32), np.asarray(b_rad, np.float32),
                            np.asarray(W_post, np.float32), np.asarray(b_post, np.float32))
    di = ex.put_inputs(in_maps)
    res = ex.run(di)
    outT_full = np.concatenate([res[c]["outT"] for c in range(N_CORES)], axis=1)
    return np.ascontiguousarray(outT_full.T)
